# revision 1
# baseline (speedup 1.0000x reference)
"""Trainium2 Bass kernel for nn_Dis_loss_69337952026648 (segment_reduce).

Strategy:
  - Data-parallel over batch: 16 samples / 8 cores = 2 samples per core.
  - The heavy part (per-sample 17-segment sums over 512x512 pixels x 8
    channels, plus counts and masked counts) runs on device:
      * host packs, per sample, a bf16 "value" tensor [128, 2048, 10]
        (channels = 8 sim channels + ones + training_mask) and a bf16 key
        tensor [128, 2048] (pixel tags 0..16).
      * per 128-pixel chunk the device builds a bf16 onehot [128, 17] with a
        DVE iota-compare and issues a PE matmul (stationary=onehot,
        moving=values) accumulating into a [17, 10] PSUM tile per sample.
  - The remaining work (17x17 pairwise distances -> scalar loss) is ~10k
    flops; done on host in float32 mirroring the reference exactly.

Counts and masked counts are exact (integer sums in fp32 PSUM), so the
discrete `present` mask matches the reference bit-exactly; only the segment
sums carry bf16 rounding (~1e-3 relative on means, ~1e-5 on the final loss).
"""

import numpy as np

B, C, H, W = 16, 8, 512, 512
NSEG = 17
NCORES = 8
SPC = B // NCORES  # samples per core
P = 128
PIX = H * W
NCHUNK = PIX // P  # 2048
NCH = C + 2  # 8 sim channels + ones + mask
GROUP = 512  # chunks per DMA/onehot group
NGROUPS = NCHUNK // GROUP
LGG_VALUE = 3.0

_CACHE = {}


def _build_nc():
    """Build + compile the Bass module (cached)."""
    if "nc" in _CACHE:
        return _CACHE["nc"]
    import sys

    if "/opt/trn_rl_repo" not in sys.path:
        sys.path.append("/opt/trn_rl_repo")
    from contextlib import ExitStack

    from concourse import bacc, mybir, tile

    nc = bacc.Bacc("TRN2", target_bir_lowering=False, debug=False)
    v_in = nc.dram_tensor(
        "v", [SPC, P, NCHUNK, NCH], mybir.dt.bfloat16, kind="ExternalInput"
    )
    gk_in = nc.dram_tensor(
        "gk", [SPC, P, NCHUNK], mybir.dt.bfloat16, kind="ExternalInput"
    )
    out = nc.dram_tensor(
        "o", [SPC, NSEG, NCH], mybir.dt.float32, kind="ExternalOutput"
    )

    with tile.TileContext(nc) as tc:
        with ExitStack() as ctx:
            const = ctx.enter_context(tc.tile_pool(name="const", bufs=1))
            vpool = ctx.enter_context(tc.tile_pool(name="v", bufs=3))
            gkpool = ctx.enter_context(tc.tile_pool(name="g", bufs=3))
            ohpool = ctx.enter_context(tc.tile_pool(name="oh", bufs=3))
            psum = ctx.enter_context(tc.tile_pool(name="ps", bufs=2, space="PSUM"))
            outpool = ctx.enter_context(tc.tile_pool(name="out", bufs=2))

            iota_i = const.tile([P, NSEG], mybir.dt.int32)
            nc.gpsimd.iota(iota_i[:], pattern=[[1, NSEG]], base=0, channel_multiplier=0)
            iota_bf = const.tile([P, NSEG], mybir.dt.bfloat16)
            nc.vector.tensor_copy(out=iota_bf[:], in_=iota_i[:])

            for s in range(SPC):
                acc = psum.tile([NSEG, NCH], mybir.dt.float32)
                for g in range(NGROUPS):
                    sl = slice(g * GROUP, (g + 1) * GROUP)
                    vt = vpool.tile([P, GROUP, NCH], mybir.dt.bfloat16)
                    nc.sync.dma_start(out=vt[:], in_=v_in[s, :, sl, :])
                    gt = gkpool.tile([P, GROUP], mybir.dt.bfloat16)
                    nc.sync.dma_start(out=gt[:], in_=gk_in[s, :, sl])
                    oh = ohpool.tile([P, GROUP, NSEG], mybir.dt.bfloat16)
                    nc.vector.tensor_tensor(
                        out=oh[:],
                        in0=gt[:, :, None].to_broadcast([P, GROUP, NSEG]),
                        in1=iota_bf[:, None, :].to_broadcast([P, GROUP, NSEG]),
                        op=mybir.AluOpType.is_equal,
                    )
                    for j in range(GROUP):
                        nc.tensor.matmul(
                            out=acc[:],
                            lhsT=oh[:, j, :],
                            rhs=vt[:, j, :],
                            start=(g == 0 and j == 0),
                            stop=(g == NGROUPS - 1 and j == GROUP - 1),
                        )
                ot = outpool.tile([NSEG, NCH], mybir.dt.float32)
                nc.vector.tensor_copy(out=ot[:], in_=acc[:])
                nc.sync.dma_start(out=out[s], in_=ot[:])

    nc.compile()
    _CACHE["nc"] = nc
    return nc


def _pack_inputs(gt_kernel_key, training_mask, similarity_vector):
    """Host-side packing into per-core device input maps."""
    import ml_dtypes

    bf16 = ml_dtypes.bfloat16
    sim = np.asarray(similarity_vector, dtype=np.float32)
    gk = np.asarray(gt_kernel_key)
    tm = np.asarray(training_mask)

    # V[b, p, j, ch]: pixel = j*128 + p
    V = np.empty((B, P, NCHUNK, NCH), dtype=bf16)
    V[..., :C] = sim.reshape(B, C, NCHUNK, P).transpose(0, 3, 2, 1).astype(bf16)
    V[..., C] = np.asarray(1.0, dtype=bf16)  # ones channel -> counts
    V[..., C + 1] = tm.reshape(B, NCHUNK, P).transpose(0, 2, 1).astype(bf16)
    GK = gk.reshape(B, NCHUNK, P).transpose(0, 2, 1).astype(bf16)

    in_maps = []
    for c in range(NCORES):
        sl = slice(c * SPC, (c + 1) * SPC)
        in_maps.append(
            {"v": np.ascontiguousarray(V[sl]), "gk": np.ascontiguousarray(GK[sl])}
        )
    return in_maps


def _loss_from_stats(stats):
    """stats: [B, 17, 10] float32 segment sums -> scalar loss (mirrors ref)."""
    sums = stats[:, :, :C]
    counts = stats[:, :, C]
    masked = stats[:, :, C + 1]
    means = sums / np.maximum(counts, 1.0)[:, :, None]
    present = masked > 0
    present[:, 0] = False
    diff = means[:, :, None, :] - means[:, None, :, :]
    dist = np.sqrt((diff * diff).sum(-1, dtype=np.float32) + np.float32(1e-12))
    pair = np.log(np.maximum(np.float32(LGG_VALUE) - dist, 0.0) ** 2 + 1.0)
    valid = present[:, :, None] & present[:, None, :] & ~np.eye(NSEG, dtype=bool)
    n_valid = valid.sum((1, 2)).astype(np.float32)
    losses = np.where(valid, pair, 0.0).sum((1, 2), dtype=np.float32) / np.maximum(
        n_valid, 1.0
    )
    sample_valid = (present.sum(1) >= 2).astype(np.float32)
    n = sample_valid.sum()
    total = (losses * sample_valid).sum(dtype=np.float32)
    out = total / max(n, np.float32(1.0)) if n > 0 else np.float32(0.0)
    return np.array(out, dtype=np.float32)


def _run_device(in_maps, trace=False, tmpdir=None):
    import sys

    if "/opt/trn_rl_repo" not in sys.path:
        sys.path.append("/opt/trn_rl_repo")
    from concourse.bass_utils import run_bass_kernel_spmd

    nc = _build_nc()
    kwargs = {}
    if trace:
        kwargs = {"trace": True, "tmpdir": tmpdir}
    return run_bass_kernel_spmd(nc, in_maps, core_ids=list(range(NCORES)), **kwargs)


def kernel(gt_kernel_key, training_mask, similarity_vector):
    in_maps = _pack_inputs(gt_kernel_key, training_mask, similarity_vector)
    res = _run_device(in_maps)
    stats = np.concatenate(
        [np.asarray(res.results[c]["o"], dtype=np.float32) for c in range(NCORES)],
        axis=0,
    )
    return _loss_from_stats(stats)


# revision 6
# speedup vs baseline: 1.0353x; 1.0353x over previous
"""Trainium2 Bass kernel for nn_Dis_loss_69337952026648 (segment_reduce).

Strategy:
  - Data-parallel over batch: 16 samples / 8 cores = 2 samples per core.
  - The heavy part (per-sample 17-segment sums over 512x512 pixels x 8
    channels, plus counts and masked counts) runs on device:
      * host packs, per sample, a bf16 "value" tensor [128, 2048, 10]
        (channels = 8 sim channels + ones + training_mask) and a bf16 key
        tensor [128, 2048] (pixel tags 0..16).
      * per 128-pixel chunk the device builds a bf16 onehot [128, 17] with a
        DVE iota-compare and issues a PE matmul (stationary=onehot,
        moving=values) accumulating into a [17, 10] PSUM tile per sample.
  - The remaining work (17x17 pairwise distances -> scalar loss) is ~10k
    flops; done on host in float32 mirroring the reference exactly.

Counts and masked counts are exact (integer sums in fp32 PSUM), so the
discrete `present` mask matches the reference bit-exactly; only the segment
sums carry bf16 rounding (~1e-3 relative on means, ~1e-5 on the final loss).
"""

import numpy as np

B, C, H, W = 16, 8, 512, 512
NSEG = 17
NCORES = 8
SPC = B // NCORES  # samples per core
P = 128
PIX = H * W
NCHUNK = PIX // P  # 2048
NCH = C + 2  # 8 sim channels + ones + mask
GROUP = 256  # chunks per DMA/onehot group
NGROUPS = NCHUNK // GROUP
LGG_VALUE = 3.0

_CACHE = {}


def _build_nc():
    """Build + compile the Bass module (cached)."""
    if "nc" in _CACHE:
        return _CACHE["nc"]
    import sys

    if "/opt/trn_rl_repo" not in sys.path:
        sys.path.append("/opt/trn_rl_repo")
    from contextlib import ExitStack

    from concourse import bacc, mybir, tile

    nc = bacc.Bacc("TRN2", target_bir_lowering=False, debug=False)
    v_in = nc.dram_tensor(
        "v", [SPC, P, NCHUNK, NCH], mybir.dt.bfloat16, kind="ExternalInput"
    )
    gk_in = nc.dram_tensor(
        "gk", [SPC, P, NCHUNK], mybir.dt.bfloat16, kind="ExternalInput"
    )
    out = nc.dram_tensor(
        "o", [SPC, NCH, NSEG], mybir.dt.float32, kind="ExternalOutput"
    )

    with tile.TileContext(nc) as tc:
        with ExitStack() as ctx:
            const = ctx.enter_context(tc.tile_pool(name="const", bufs=1))
            vpool = ctx.enter_context(tc.tile_pool(name="v", bufs=3))
            gkpool = ctx.enter_context(tc.tile_pool(name="g", bufs=3))
            ohpool = ctx.enter_context(tc.tile_pool(name="oh", bufs=3))
            psum = ctx.enter_context(tc.tile_pool(name="ps", bufs=2, space="PSUM"))
            outpool = ctx.enter_context(tc.tile_pool(name="out", bufs=2))

            iota_i = const.tile([P, NSEG], mybir.dt.int32)
            nc.gpsimd.iota(iota_i[:], pattern=[[1, NSEG]], base=0, channel_multiplier=0)
            iota_bf = const.tile([P, NSEG], mybir.dt.bfloat16)
            nc.vector.tensor_copy(out=iota_bf[:], in_=iota_i[:])

            for s in range(SPC):
                acc = psum.tile([NCH, NSEG], mybir.dt.float32)
                for g in range(NGROUPS):
                    sl = slice(g * GROUP, (g + 1) * GROUP)
                    vt = vpool.tile([P, GROUP, NCH], mybir.dt.bfloat16)
                    nc.sync.dma_start(out=vt[:], in_=v_in[s, :, sl, :])
                    gt = gkpool.tile([P, GROUP], mybir.dt.bfloat16)
                    nc.sync.dma_start(out=gt[:], in_=gk_in[s, :, sl])
                    oh = ohpool.tile([P, GROUP, NSEG], mybir.dt.bfloat16)
                    nc.vector.tensor_tensor(
                        out=oh[:],
                        in0=gt[:, :, None].to_broadcast([P, GROUP, NSEG]),
                        in1=iota_bf[:, None, :].to_broadcast([P, GROUP, NSEG]),
                        op=mybir.AluOpType.is_equal,
                    )
                    for j in range(GROUP):
                        # stationary = V chunk (10 cols -> cheap LDWEIGHTS),
                        # moving = onehot (N=17); out = [NCH, NSEG]
                        nc.tensor.matmul(
                            out=acc[:],
                            lhsT=vt[:, j, :],
                            rhs=oh[:, j, :],
                            start=(g == 0 and j == 0),
                            stop=(g == NGROUPS - 1 and j == GROUP - 1),
                        )
                ot = outpool.tile([NCH, NSEG], mybir.dt.float32)
                nc.vector.tensor_copy(out=ot[:], in_=acc[:])
                nc.sync.dma_start(out=out[s], in_=ot[:])

    nc.compile()
    _CACHE["nc"] = nc
    return nc


def _pack_inputs(gt_kernel_key, training_mask, similarity_vector):
    """Host-side packing into per-core device input maps."""
    import ml_dtypes

    bf16 = ml_dtypes.bfloat16
    sim = np.asarray(similarity_vector, dtype=np.float32)
    gk = np.asarray(gt_kernel_key)
    tm = np.asarray(training_mask)

    # V[b, p, j, ch]: pixel = j*128 + p
    V = np.empty((B, P, NCHUNK, NCH), dtype=bf16)
    V[..., :C] = sim.reshape(B, C, NCHUNK, P).transpose(0, 3, 2, 1).astype(bf16)
    V[..., C] = np.asarray(1.0, dtype=bf16)  # ones channel -> counts
    V[..., C + 1] = tm.reshape(B, NCHUNK, P).transpose(0, 2, 1).astype(bf16)
    GK = gk.reshape(B, NCHUNK, P).transpose(0, 2, 1).astype(bf16)

    in_maps = []
    for c in range(NCORES):
        sl = slice(c * SPC, (c + 1) * SPC)
        in_maps.append(
            {"v": np.ascontiguousarray(V[sl]), "gk": np.ascontiguousarray(GK[sl])}
        )
    return in_maps


def _loss_from_stats(stats):
    """stats: [B, 10, 17] float32 segment sums -> scalar loss (mirrors ref)."""
    stats = stats.transpose(0, 2, 1)  # -> [B, 17, 10]
    sums = stats[:, :, :C]
    counts = stats[:, :, C]
    masked = stats[:, :, C + 1]
    means = sums / np.maximum(counts, 1.0)[:, :, None]
    present = masked > 0
    present[:, 0] = False
    diff = means[:, :, None, :] - means[:, None, :, :]
    dist = np.sqrt((diff * diff).sum(-1, dtype=np.float32) + np.float32(1e-12))
    pair = np.log(np.maximum(np.float32(LGG_VALUE) - dist, 0.0) ** 2 + 1.0)
    valid = present[:, :, None] & present[:, None, :] & ~np.eye(NSEG, dtype=bool)
    n_valid = valid.sum((1, 2)).astype(np.float32)
    losses = np.where(valid, pair, 0.0).sum((1, 2), dtype=np.float32) / np.maximum(
        n_valid, 1.0
    )
    sample_valid = (present.sum(1) >= 2).astype(np.float32)
    n = sample_valid.sum()
    total = (losses * sample_valid).sum(dtype=np.float32)
    out = total / max(n, np.float32(1.0)) if n > 0 else np.float32(0.0)
    return np.array(out, dtype=np.float32)


def _run_device(in_maps, trace=False, tmpdir=None):
    import sys

    if "/opt/trn_rl_repo" not in sys.path:
        sys.path.append("/opt/trn_rl_repo")
    from concourse.bass_utils import run_bass_kernel_spmd

    nc = _build_nc()
    kwargs = {}
    if trace:
        kwargs = {"trace": True, "tmpdir": tmpdir}
    return run_bass_kernel_spmd(nc, in_maps, core_ids=list(range(NCORES)), **kwargs)


def kernel(gt_kernel_key, training_mask, similarity_vector):
    in_maps = _pack_inputs(gt_kernel_key, training_mask, similarity_vector)
    res = _run_device(in_maps)
    stats = np.concatenate(
        [np.asarray(res.results[c]["o"], dtype=np.float32) for c in range(NCORES)],
        axis=0,
    )
    return _loss_from_stats(stats)


# revision 7
# speedup vs baseline: 1.7431x; 1.6836x over previous
"""Trainium2 Bass kernel for nn_Dis_loss_69337952026648 (segment_reduce).

Strategy (v2 — fp8 DoubleRow):
  - Data-parallel over batch: 16 samples / 8 cores = 2 samples per core.
  - Per sample we need 16-segment sums over 512x512 pixels of 10 values
    (8 sim channels + ones->counts + mask->masked counts). Tag 0
    (background) never contributes to the loss (present[0] is forced False
    in the reference), so only tags 1..16 are reduced -> M=16.
  - Host precomputes, per sample:
      * onehot fp8e4m3 [128, 1024, 2, 16]  (pixel q = j*256 + u*128 + p)
      * values fp8e4m3 [128, 1024, 2, 10]
  - Device: per j-chunk one PE matmul in DoubleRow perf mode
    (K=256 pixels per instruction): lhsT = onehot [128, 2, 16],
    rhs = values [128, 2, 10], accumulated into a [16, 10] fp32 PSUM tile
    per sample. 1024 matmuls per sample, 2048 per core.
  - Host finishes the tiny pairwise-distance loss in float32, mirroring the
    reference exactly.

Exactness notes: ones/mask/onehot are 0/1 (exact in fp8); PSUM accumulates
in fp32, so counts and masked counts are exact integers and the discrete
`present` mask matches the reference bit-exactly. Only the sim segment sums
carry fp8 rounding (~3% on per-tag means, which the loss smooths to ~1e-4).
"""

import numpy as np

B, C, H, W = 16, 8, 512, 512
NSEG = 17
NTAG = 16  # tags 1..16 (tag 0 dropped)
NCORES = 8
SPC = B // NCORES  # samples per core
P = 128
PIX = H * W
NCHUNK2 = PIX // (2 * P)  # 1024 double-row chunks
NCH = C + 2  # 8 sim channels + ones + mask
GROUP = 256  # j-chunks per DMA group
NGROUPS = NCHUNK2 // GROUP
LGG_VALUE = 3.0

_CACHE = {}


def _build_nc():
    """Build + compile the Bass module (cached)."""
    if "nc" in _CACHE:
        return _CACHE["nc"]
    import sys

    if "/opt/trn_rl_repo" not in sys.path:
        sys.path.append("/opt/trn_rl_repo")
    from contextlib import ExitStack

    from concourse import bacc, mybir, tile

    nc = bacc.Bacc("TRN2", target_bir_lowering=False, debug=False)
    v_in = nc.dram_tensor(
        "v", [SPC, P, NCHUNK2, 2, NCH], mybir.dt.float8e4, kind="ExternalInput"
    )
    oh_in = nc.dram_tensor(
        "oh", [SPC, P, NCHUNK2, 2, NTAG], mybir.dt.float8e4, kind="ExternalInput"
    )
    out = nc.dram_tensor(
        "o", [SPC, NTAG, NCH], mybir.dt.float32, kind="ExternalOutput"
    )

    with tile.TileContext(nc) as tc:
        with ExitStack() as ctx:
            vpool = ctx.enter_context(tc.tile_pool(name="v", bufs=3))
            ohpool = ctx.enter_context(tc.tile_pool(name="oh", bufs=3))
            psum = ctx.enter_context(tc.tile_pool(name="ps", bufs=2, space="PSUM"))
            outpool = ctx.enter_context(tc.tile_pool(name="out", bufs=2))

            for s in range(SPC):
                acc = psum.tile([NTAG, NCH], mybir.dt.float32)
                for g in range(NGROUPS):
                    sl = slice(g * GROUP, (g + 1) * GROUP)
                    vt = vpool.tile([P, GROUP, 2, NCH], mybir.dt.float8e4)
                    nc.sync.dma_start(out=vt[:], in_=v_in[s, :, sl, :, :])
                    oht = ohpool.tile([P, GROUP, 2, NTAG], mybir.dt.float8e4)
                    nc.sync.dma_start(out=oht[:], in_=oh_in[s, :, sl, :, :])
                    for j in range(GROUP):
                        nc.tensor.matmul(
                            out=acc[:],
                            lhsT=oht[:, j, :, :],
                            rhs=vt[:, j, :, :],
                            start=(g == 0 and j == 0),
                            stop=(g == NGROUPS - 1 and j == GROUP - 1),
                            perf_mode=mybir.MatmulPerfMode.DoubleRow,
                        )
                ot = outpool.tile([NTAG, NCH], mybir.dt.float32)
                nc.vector.tensor_copy(out=ot[:], in_=acc[:])
                nc.sync.dma_start(out=out[s], in_=ot[:])

    nc.compile()
    _CACHE["nc"] = nc
    return nc


def _pack_inputs(gt_kernel_key, training_mask, similarity_vector):
    """Host-side packing into per-core device input maps."""
    import ml_dtypes

    fp8 = ml_dtypes.float8_e4m3
    sim = np.asarray(similarity_vector, dtype=np.float32)
    gk = np.asarray(gt_kernel_key)
    tm = np.asarray(training_mask)

    # pixel q = j*256 + u*128 + p  ->  [b, p, j, u]
    # V[b, p, j, u, ch]
    V = np.empty((B, P, NCHUNK2, 2, NCH), dtype=fp8)
    V[..., :C] = (
        sim.reshape(B, C, NCHUNK2, 2, P).transpose(0, 4, 2, 3, 1).astype(fp8)
    )
    V[..., C] = np.asarray(1.0, dtype=fp8)  # ones channel -> counts
    V[..., C + 1] = tm.reshape(B, NCHUNK2, 2, P).transpose(0, 3, 1, 2).astype(fp8)

    # onehot over tags 1..16 via lookup table
    lut = np.zeros((NSEG, NTAG), dtype=fp8)
    for t in range(1, NSEG):
        lut[t, t - 1] = 1.0
    gkp = gk.reshape(B, NCHUNK2, 2, P).transpose(0, 3, 1, 2)  # [b, p, j, u]
    OH = lut[gkp]  # [b, p, j, u, 16]

    in_maps = []
    for c in range(NCORES):
        sl = slice(c * SPC, (c + 1) * SPC)
        in_maps.append(
            {"v": np.ascontiguousarray(V[sl]), "oh": np.ascontiguousarray(OH[sl])}
        )
    return in_maps


def _loss_from_stats(stats):
    """stats: [B, 16, 10] float32 segment sums (tags 1..16) -> scalar loss."""
    sums = stats[:, :, :C]
    counts = stats[:, :, C]
    masked = stats[:, :, C + 1]
    means = sums / np.maximum(counts, 1.0)[:, :, None]
    present = masked > 0  # [B, 16]
    diff = means[:, :, None, :] - means[:, None, :, :]
    dist = np.sqrt((diff * diff).sum(-1, dtype=np.float32) + np.float32(1e-12))
    pair = np.log(np.maximum(np.float32(LGG_VALUE) - dist, 0.0) ** 2 + 1.0)
    valid = present[:, :, None] & present[:, None, :] & ~np.eye(NTAG, dtype=bool)
    n_valid = valid.sum((1, 2)).astype(np.float32)
    losses = np.where(valid, pair, 0.0).sum((1, 2), dtype=np.float32) / np.maximum(
        n_valid, 1.0
    )
    sample_valid = (present.sum(1) >= 2).astype(np.float32)
    n = sample_valid.sum()
    total = (losses * sample_valid).sum(dtype=np.float32)
    out = total / max(n, np.float32(1.0)) if n > 0 else np.float32(0.0)
    return np.array(out, dtype=np.float32)


def _run_device(in_maps, trace=False, tmpdir=None):
    import sys

    if "/opt/trn_rl_repo" not in sys.path:
        sys.path.append("/opt/trn_rl_repo")
    from concourse.bass_utils import run_bass_kernel_spmd

    nc = _build_nc()
    kwargs = {}
    if trace:
        kwargs = {"trace": True, "tmpdir": tmpdir}
    return run_bass_kernel_spmd(nc, in_maps, core_ids=list(range(NCORES)), **kwargs)


def kernel(gt_kernel_key, training_mask, similarity_vector):
    in_maps = _pack_inputs(gt_kernel_key, training_mask, similarity_vector)
    res = _run_device(in_maps)
    stats = np.concatenate(
        [np.asarray(res.results[c]["o"], dtype=np.float32) for c in range(NCORES)],
        axis=0,
    )
    return _loss_from_stats(stats)
